# revision 14
# baseline (speedup 1.0000x reference)
"""Embedding lookup (gather) on 8 TRN2 NeuronCores.

Strategy: replicate the 1M x 128 fp32 table to every core's HBM and shard the
500K indices 8 ways. Each core gathers its 62.5K rows locally with a handful
of large indirect (SWDGE) DMAs -- no collectives needed. Per core the traffic
is 32MB gathered reads + 32MB contiguous writes, against a ~360 GB/s DMA bus.

Index layout per core: the 62592 (padded) indices are reshaped row-major to
[128, 489] so that one indirect_dma_start with offset AP [128, C] gathers
128*C rows in a single instruction (row (p,c) -> output row p*489+c), and the
write-back to DRAM is one fully-contiguous strided DMA per chunk.
"""
import sys
import numpy as np

sys.path.insert(0, "/opt/trn_rl_repo")

import concourse.bacc as bacc
import concourse.bass as bass
import concourse.mybir as mybir
import concourse.tile as tile
from concourse import bass_utils

N_EMB = 1_000_000
D = 128
N_IDX = 500_000
N_CORES = 8

P = 128                      # SBUF partitions
COLS = 489                   # index columns per partition: 128*489 = 62592 rows/core
ROWS_PER_CORE = P * COLS     # 62592
PAD_TOTAL = N_CORES * ROWS_PER_CORE  # 500736

# Column chunks per indirect-DMA instruction. C*D*4 bytes/partition of SBUF
# per buffer; the gather's per-partition extent C*512B must fit a 16-bit ISA
# field, so C <= 127. 123*512B = ~63KB, x2 bufs fits the 192KB/partition budget.
CHUNKS = []
_c0 = 0
for _C in (123, 122, 122, 122):
    CHUNKS.append((_c0, _C))
    _c0 += _C
assert _c0 == COLS

_cached = None


def _build():
    global _cached
    if _cached is not None:
        return _cached

    nc = bacc.Bacc(
        "TRN2",
        target_bir_lowering=False,
        debug=False,
        enable_asserts=False,
        num_devices=N_CORES,
    )
    idx_dram = nc.dram_tensor(
        "idx", [P, COLS], mybir.dt.int32, kind="ExternalInput"
    ).ap()
    weight = nc.dram_tensor(
        "weight", [N_EMB, D], mybir.dt.float32, kind="ExternalInput"
    ).ap()
    out = nc.dram_tensor(
        "out", [P, COLS * D], mybir.dt.float32, kind="ExternalOutput"
    ).ap()

    with tile.TileContext(nc) as tc:
        with (
            tc.tile_pool(name="idxp", bufs=1) as idxp,
            tc.tile_pool(name="pool", bufs=2) as pool,
        ):
            idx_all = idxp.tile([P, COLS], mybir.dt.int32)
            nc.sync.dma_start(out=idx_all[:, :], in_=idx_dram[:, :])
            for c0, C in CHUNKS:
                g = pool.tile([P, C * D], mybir.dt.float32, tag="g")
                # One indirect DMA per index column: the HW SWDGE ucode uses
                # ONE index per partition per instruction (transfers the whole
                # per-partition dest extent contiguously), so a multi-column
                # offset AP silently gathers wrong rows. [P,1] offsets are the
                # production-proven idiom.
                for c in range(C):
                    nc.gpsimd.indirect_dma_start(
                        out=g[:, c * D:(c + 1) * D],
                        out_offset=None,
                        in_=weight[:],
                        in_offset=bass.IndirectOffsetOnAxis(
                            ap=idx_all[:, c0 + c:c0 + c + 1], axis=0
                        ),
                    )
                nc.sync.dma_start(out=out[:, c0 * D:(c0 + C) * D], in_=g[:])

    nc.compile()
    _cached = nc
    return nc


def kernel(input, weight, _trace=False, _tmpdir=None):
    nc = _build()

    idx = np.ascontiguousarray(np.asarray(input).astype(np.int32))
    w = np.ascontiguousarray(np.asarray(weight, dtype=np.float32))

    idx_pad = np.zeros(PAD_TOTAL, dtype=np.int32)
    idx_pad[:N_IDX] = idx
    idx_cores = idx_pad.reshape(N_CORES, P, COLS)

    in_maps = [{"idx": idx_cores[c], "weight": w} for c in range(N_CORES)]

    res = bass_utils.run_bass_kernel_spmd(
        nc,
        in_maps,
        core_ids=list(range(N_CORES)),
        trace=_trace,
        tmpdir=_tmpdir,
    )

    out = np.concatenate(
        [res.results[c]["out"].reshape(ROWS_PER_CORE, D) for c in range(N_CORES)],
        axis=0,
    )[:N_IDX]
    if _trace:
        return out, res
    return out
